# revision 66
# baseline (speedup 1.0000x reference)
"""Bass/Trainium2 kernel for BoundaryAwareDownConv.

Computation (see reference): for x[B=8, T=8192, D=512] with a space token at
every position t % 8 == 7, pool each 8-token segment by the mean of its 7
non-space tokens -> pooled[B, W=1024, D], then proj = pooled @ w_proj.T +
b_proj, then LayerNorm(D) * gamma + beta.

Sharding: data-parallel over batch, one batch row per NeuronCore (8 cores).
Params are replicated.

Final pipeline (per core; ~46-49 us vs the 112-121 us accumulate-DMA
baseline):
  - x is TRANSPOSED AND DE-SPACED ON THE HOST into
    xH[128 part, 8 chunk, 4 dblk, 7 j, 128 seg] fp16 with d = 128*dblk +
    part and token = 1024*chunk + 8*seg + j: the device never transposes
    anything, the space tokens are never staged (12.5% less HBM traffic),
    and fp16 staging halves it again. Each chunk DMA reads one contiguous
    7 KB descriptor per partition at line rate.
  - Chunk loads ALTERNATE between the two HWDGE rings (SP/ACT) and the
    mid-stream chunks (2,3) and (4,5) are GROUPED into single double-chunk
    DMAs (one contiguous 14 KB descriptor per partition): the tile
    scheduler serializes each ring's per-dma_start fixed cost (~2 us), so
    fewer+bigger mid-stream DMAs cut ring overhead while the fill (0,1)
    and tail (6,7) chunks stay single for minimal latency. Param loads
    lead the HWDGE rings (tiny bias/ones ahead of x on SP, wt ahead of x
    on ACT) - on the SWDGE ring they starve behind the constant HWDGE
    packet traffic and stalled the PE's first matmul by ~17 us.
  - Pooling per chunk is a 4-op fp16 pair-add tree on DVE (contiguous
    128-seg inner runs keep the 16-bit 2-elem/cycle mode) producing
    pooledT[128 dk, 4 blk, 128 seg] directly; the space row is simply
    never added. Scale 1/7 is folded into the staged w.
  - The staged projection weights are CENTERED host-side (wT - rowmean,
    b - mean(b)): the matmul produces proj - mean(proj) exactly (the LN
    mean is linear in the weights), so no mean/bn_stats pass runs on
    device.
  - PE per chunk: K=1 ones x bias_row matmul (bias lives inside the PSUM
    accumulation group - no extra engine hop) + 4 K=128 fp16 matmuls
    pooledT.T @ wTc into PSUM f32.
  - LayerNorm: sum of squares via one ACT Square pass with row-accumulate,
    Sqrt(ss/512+eps) on ACT, reciprocal on DVE (one chunk later, off the
    critical chain), scale-only apply on ACT, fp16 out tile, upcast to f32
    on the host. Out stores are batched per chunk pair on the SP ring
    (after all loads in SP program order - no head-of-line blocking).
  - Emission is software-pipelined in three stages (load+pool / proj+stats
    / apply+store) so late-stage ops never block a later chunk's
    early-stage ops in engine program order.
"""

import numpy as np

B, T, D = 8, 8192, 512
STRIDE = 8
W = T // STRIDE  # 1024
LN_EPS = 1e-5
N_CORES = 8
N_CHUNKS = 8         # 128 segments (= 1024 tokens) per chunk
VALID = STRIDE - 1   # 7 non-space tokens per segment


def _patched_tile_context(tile, mybir, ScopedClock):
    """TileContext whose kernel-tail drain carries no sem waits.

    The walrus build in this container rejects sync-wait commands on Drain
    instructions (setupSyncWait<...NO_STRUCT>: "Too many sync wait commands").
    Stock TileContext parks the global-clock catch-up waits on the SP Drain;
    park them on SP nops (one wait each) instead.
    """

    class PatchedTileContext(tile.TileContext):
        def _drain_and_barrier(self, tick_clock, wait_clock):
            required = ScopedClock({None: tick_clock.global_clock})
            carrier = self.nc.sync.nop(nofuse=True)
            wait_clock.add_sem_waits(carrier.ins, required)
            si = carrier.ins.sync_info
            waits = list(si.on_wait) if si is not None else []
            if len(waits) > 1:
                si.on_wait = waits[:1]
                carrier.ins.sync_info = si
                for w in waits[1:]:
                    extra = self.nc.sync.nop(nofuse=True)
                    extra.ins.sync_info = mybir.SyncInfo(on_wait=[w], on_update=[])
            # The carrier nops run earlier on the same (SP) engine, so the
            # drain transitively waits on everything without carrying waits.
            self.nc.sync.drain()
            self.nc.all_engine_barrier()
            assert self.sems is not None
            popped = self.nc._tile_sem_poison_stack.pop()
            assert popped is self._sem_poison
            self.nc.clear_and_free_semaphores(list(self.sems.allocated().values()))
            self.nc.all_engine_barrier()

    return PatchedTileContext


def _split_multi_waits(nc, mybir):
    """Rewrite the scheduled BIR so no instruction carries more than one sync
    wait (and Drain carries none): the walrus build here rejects them
    (setupSyncWait: "Too many sync wait commands"). Surplus waits move onto
    same-engine InstNoOp carriers placed immediately before the instruction -
    same-engine program order preserves the blocking semantics."""
    n = 0
    for fn in nc.m.functions:
        for bb in fn.blocks:
            changed = False
            new_insts = []
            for inst in bb.instructions:
                si = inst.sync_info
                waits = list(si.on_wait) if si is not None else []
                limit = 0 if inst.opcode == "Drain" else 1
                if len(waits) > limit:
                    changed = True
                    for w in waits[limit:]:
                        n += 1
                        new_insts.append(
                            mybir.InstNoOp(
                                name=f"wsplit_{n}_{inst.name}",
                                engine=inst.engine,
                                sync_info=mybir.SyncInfo(on_wait=[w], on_update=[]),
                                bass_nofuse=True,
                            )
                        )
                    si.on_wait = waits[:limit]
                    inst.sync_info = si
                new_insts.append(inst)
            if changed:
                bb.instructions = new_insts


def _build_bass(apply_gamma_beta: bool, split_waits: bool = True):
    import concourse.bass as bass
    import concourse.mybir as mybir
    import concourse.tile as tile
    from concourse.bass import ts
    from concourse.vector_clock import ScopedClock

    PatchedTileContext = _patched_tile_context(tile, mybir, ScopedClock)
    f32 = mybir.dt.float32
    f16 = mybir.dt.float16

    nc = bass.Bass("TRN2")
    # host-transposed x: [part, chunk, dblk, j, seg] with d = 128*dblk + part,
    # token = 1024*chunk + 8*seg + j (contiguous 128-seg inner runs keep the
    # DVE pooling adds in 16-bit 2x mode)
    x = nc.dram_tensor(
        "x", [128, N_CHUNKS, 4, VALID, 128], f16, kind="ExternalInput"
    )
    # w_proj.T / 7, centered over dout
    wT = nc.dram_tensor("wT", [D, D], f16, kind="ExternalInput")
    bias = nc.dram_tensor("bias", [1, D], f16, kind="ExternalInput")
    ones1 = nc.dram_tensor("ones1", [1, 128], f16, kind="ExternalInput")
    if apply_gamma_beta:
        gammaB = nc.dram_tensor("gammaB", [128, D], f32, kind="ExternalInput")
        betaB = nc.dram_tensor("betaB", [128, D], f32, kind="ExternalInput")
    out = nc.dram_tensor("out", [W, D], f16, kind="ExternalOutput")

    with PatchedTileContext(nc) as tc:
        with (
            tc.tile_pool(name="singles", bufs=1) as singles,
            tc.tile_pool(name="xr_pool", bufs=3) as xr_pool,
            tc.tile_pool(name="t_pool", bufs=2) as t_pool,
            tc.tile_pool(name="uv_pool", bufs=2) as uv_pool,
            tc.tile_pool(name="pmT_pool", bufs=4) as pmT_pool,
            tc.tile_pool(name="sq_pool", bufs=3) as sq_pool,
            tc.tile_pool(name="out_sb", bufs=3) as out_sb,
            tc.tile_pool(name="stat", bufs=6) as stat,
            tc.tile_pool(name="ps_proj", bufs=5, space="PSUM") as ps_proj,
        ):
            # One-time loads ride the HWDGE rings ahead of the x streams:
            # SWDGE singles starve behind constant HWDGE traffic (SDMA
            # round-robins at packet granularity), which stalled the PE's
            # first matmul ~20 us. bias/ones are tiny (SP, ~0.1 us before
            # x chunk 0); wt (512 KB) leads the ACT ring before x chunk 1.
            bias_sb = singles.tile([1, D], f16)
            nc.sync.dma_start(out=bias_sb[:], in_=bias[:, :])
            ones_sb = singles.tile([1, 128], f16)
            nc.sync.dma_start(out=ones_sb[:], in_=ones1[:, :])
            wt_sb = singles.tile([128, 4, D], f16)  # [d_lo, d_hi, dout]
            nc.scalar.dma_start(
                out=wt_sb[:], in_=wT[:, :].rearrange("(k p) n -> p k n", p=128)
            )
            eps_sb = singles.tile([128, 1], f32)
            nc.vector.memset(eps_sb[:], LN_EPS)
            # Warm-up: trigger the one-time ACT table load and PE state
            # load early, overlapped with the first x-chunk DMA.
            warm_sb = singles.tile([128, 1], f32)
            nc.scalar.activation(
                out=warm_sb[:],
                in_=eps_sb[:],
                func=mybir.ActivationFunctionType.Identity,
                scale=1.0,
            )
            warm_ps = ps_proj.tile([128, D], f32, name="warm", bufs=1)
            nc.tensor.matmul(
                warm_ps[:], lhsT=ones_sb[:], rhs=bias_sb[:], start=True, stop=True
            )
            if apply_gamma_beta:
                g_sb = singles.tile([128, D], f32)
                nc.scalar.dma_start(out=g_sb[:], in_=gammaB[:, :])
                b_sb = singles.tile([128, D], f32)
                nc.scalar.dma_start(out=b_sb[:], in_=betaB[:, :])

            pmTs = {}
            pps = {}
            rstds = {}

            xrs = {}

            def stage_load(group, eng):
                # Grouped loads: chunks are contiguous in xH, so a 2-chunk
                # group is ONE 14 KB descriptor per partition - half the
                # per-dma_start ring fixed cost. Fill (0,1) and tail (6,7)
                # chunks stay single so their latency is minimal.
                n = len(group)
                xr = xr_pool.tile(
                    [128, n, 4, VALID, 128], f16, name=f"xr{n}"
                )
                c0 = group[0]
                eng.dma_start(out=xr[:], in_=x[:, c0 : c0 + n, :, :, :])
                for i, R in enumerate(group):
                    xrs[R] = (xr, i)

            def stage_pool(R):
                xr, i = xrs.pop(R)
                # Pooling: fp16 pair-add tree on DVE (contiguous 128-seg
                # inner runs -> 2 elem/cycle); the space row is never added.
                with nc.allow_low_precision(reason="fp16 pooling"):
                    t = t_pool.tile([128, 4, 3, 128], f16, name="t")
                    nc.vector.tensor_add(
                        t[:], xr[:, i, :, 0:5:2, :], xr[:, i, :, 1:6:2, :]
                    )
                    uv1 = uv_pool.tile([128, 4, 128], f16, name="uv1")
                    nc.vector.tensor_add(
                        uv1[:], t[:, :, 2, :], xr[:, i, :, 6, :]
                    )
                    uv0 = uv_pool.tile([128, 4, 128], f16, name="uv0")
                    nc.vector.tensor_add(
                        uv0[:], t[:, :, 0, :], t[:, :, 1, :]
                    )
                    pmT = pmT_pool.tile([128, 4, 128], f16, name="pmT")
                    nc.vector.tensor_add(pmT[:], uv0[:], uv1[:])
                pmTs[R] = pmT

            def stage_b1(R):
                pmT = pmTs.pop(R)
                # projection for w-chunk R: psum[seg 128, dout 512]; the
                # bias lives in the accumulation group (K=1 ones x bias).
                pp = ps_proj.tile([128, D], f32, name="pp")
                nc.tensor.matmul(
                    pp[:], lhsT=ones_sb[:], rhs=bias_sb[:], start=True, stop=False
                )
                for k in range(4):
                    nc.tensor.matmul(
                        pp[:],
                        lhsT=pmT[:, k, :],
                        rhs=wt_sb[:, k, :],
                        start=False,
                        stop=(k == 3),
                    )
                # Sum of squares via one ACT Square pass with row-accumulate,
                # then Sqrt; the tiny reciprocal runs on DVE one chunk later
                # (stage_b2) so the ACT->DVE->ACT zig is off the chain.
                sq = sq_pool.tile([128, D], f16, name="sq")
                ss = stat.tile([128, 1], f32, name="ss")
                nc.scalar.activation(
                    out=sq[:],
                    in_=pp[:],
                    func=mybir.ActivationFunctionType.Square,
                    accum_out=ss[:],
                )
                rstd = stat.tile([128, 1], f32, name="rstd")
                nc.scalar.activation(
                    out=rstd[:],
                    in_=ss[:],
                    func=mybir.ActivationFunctionType.Sqrt,
                    bias=eps_sb[:],
                    scale=1.0 / D,
                )
                pps[R] = pp
                rstds[R] = rstd

            ots = {}

            def stage_b2(R):
                pp = pps.pop(R)
                rstd = rstds.pop(R)
                nc.vector.reciprocal(out=rstd[:], in_=rstd[:])
                # Output tiles are batched per chunk pair: one 256-row
                # store per two chunks halves the per-store fixed cost.
                if R % 2 == 0:
                    ot2 = out_sb.tile([128, 2, D], f16, name="ot2")
                    ots[R // 2] = ot2
                else:
                    ot2 = ots[R // 2]
                if apply_gamma_beta:
                    ot32 = out_sb.tile([128, D], f32, name="ot32")
                    nc.scalar.activation(
                        out=ot32[:],
                        in_=pp[:],
                        func=mybir.ActivationFunctionType.Identity,
                        scale=rstd[:],
                    )
                    nc.vector.tensor_mul(out=ot32[:], in0=ot32[:], in1=g_sb[:])
                    nc.vector.tensor_add(
                        out=ot2[:, R % 2, :], in0=ot32[:], in1=b_sb[:]
                    )
                else:
                    nc.scalar.activation(
                        out=ot2[:, R % 2, :],
                        in_=pp[:],
                        func=mybir.ActivationFunctionType.Identity,
                        scale=rstd[:],
                    )
                if R % 2 == 1:
                    ot2 = ots.pop(R // 2)
                    nc.sync.dma_start(
                        out=out[R // 2 * 256 : R // 2 * 256 + 256, :].rearrange(
                            "(c p) n -> p c n", p=128
                        ),
                        in_=ot2[:],
                    )

            # Software-pipelined emission: grouped loads lead, pooling
            # follows per chunk, proj+stats (b1) runs ~2 chunks behind,
            # apply+store (b2) one more behind.
            S, A = nc.sync, nc.scalar
            schedule = [
                ("l", (0,), S), ("l", (1,), A), ("l", (2, 3), S),
                ("p", 0), ("l", (4, 5), A), ("p", 1), ("b1", 0),
                ("l", (6,), S), ("p", 2), ("b1", 1), ("l", (7,), A),
                ("b2", 0), ("p", 3), ("b1", 2), ("b2", 1),
                ("p", 4), ("b1", 3), ("b2", 2),
                ("p", 5), ("b1", 4), ("b2", 3),
                ("p", 6), ("b1", 5), ("b2", 4),
                ("p", 7), ("b1", 6), ("b2", 5),
                ("b1", 7), ("b2", 6), ("b2", 7),
            ]
            for item in schedule:
                if item[0] == "l":
                    stage_load(item[1], item[2])
                elif item[0] == "p":
                    stage_pool(item[1])
                elif item[0] == "b1":
                    stage_b1(item[1])
                else:
                    stage_b2(item[1])

    if split_waits:
        _split_multi_waits(nc, mybir)
    return nc


def _stage_inputs(inputs) -> tuple[bool, list[dict]]:
    """Host-side staging: transposed fp16 x per core + replicated params."""
    x = np.asarray(inputs["x"], dtype=np.float32)
    w = np.asarray(inputs["w_proj"], dtype=np.float32)
    b = np.asarray(inputs["b_proj"], dtype=np.float32)
    gamma = np.asarray(inputs["gamma"], dtype=np.float32)
    beta = np.asarray(inputs["beta"], dtype=np.float32)
    assert x.shape == (B, T, D), x.shape

    apply_gb = not (np.all(gamma == 1.0) and np.all(beta == 0.0))
    # Center the projection over dout so the device matmul yields
    # proj - mean(proj) directly (mean is linear in the weights).
    wt = (w.T / VALID).astype(np.float64)
    wtc = wt - wt.mean(axis=1, keepdims=True)
    bc = b.astype(np.float64) - b.astype(np.float64).mean()
    common = {
        "wT": np.ascontiguousarray(wtc).astype(np.float16),
        "bias": np.ascontiguousarray(bc.reshape(1, D)).astype(np.float16),
        "ones1": np.ones((1, 128), dtype=np.float16),
    }
    if apply_gb:
        common["gammaB"] = np.ascontiguousarray(
            np.broadcast_to(gamma.reshape(1, D), (128, D))
        )
        common["betaB"] = np.ascontiguousarray(
            np.broadcast_to(beta.reshape(1, D), (128, D))
        )
    x16 = x.astype(np.float16)
    # xH[p, c, k, j, s] = x[c*1024 + 8*s + j, 128*k + p], j < 7 only
    # (space tokens are never staged).
    in_maps = []
    for i in range(N_CORES):
        xh = np.ascontiguousarray(
            x16[i]
            .reshape(N_CHUNKS, 128, STRIDE, 4, 128)[:, :, :VALID, :, :]
            .transpose(4, 0, 3, 2, 1)
        )
        in_maps.append({"x": xh, **common})
    return apply_gb, in_maps


def kernel(**inputs) -> np.ndarray:
    from concourse.bass_utils import run_bass_kernel_spmd

    apply_gb, in_maps = _stage_inputs(inputs)
    nc = _build_bass(apply_gb)
    res = run_bass_kernel_spmd(nc, in_maps, core_ids=list(range(N_CORES)))
    return np.stack(
        [res.results[i]["out"].astype(np.float32) for i in range(N_CORES)], axis=0
    )


if __name__ == "__main__":
    rng = np.random.default_rng(0)
    demo = {
        "x": rng.standard_normal((B, T, D), dtype=np.float32),
        "input_ids": np.zeros((B, T), dtype=np.int64),
        "w_proj": rng.standard_normal((D, D), dtype=np.float32) / np.sqrt(D),
        "b_proj": (rng.standard_normal(D) * 0.01).astype(np.float32),
        "gamma": np.ones(D, dtype=np.float32),
        "beta": np.zeros(D, dtype=np.float32),
    }
    out = kernel(**demo)
    print(out.shape, out.dtype, float(np.abs(out).mean()))


# revision 67
# speedup vs baseline: 1.1530x; 1.1530x over previous
"""Bass/Trainium2 kernel for BoundaryAwareDownConv.

Computation (see reference): for x[B=8, T=8192, D=512] with a space token at
every position t % 8 == 7, pool each 8-token segment by the mean of its 7
non-space tokens -> pooled[B, W=1024, D], then proj = pooled @ w_proj.T +
b_proj, then LayerNorm(D) * gamma + beta.

Sharding: data-parallel over batch, one batch row per NeuronCore (8 cores).
Params are replicated.

Final pipeline (per core; ~46-49 us vs the 112-121 us accumulate-DMA
baseline):
  - x is TRANSPOSED AND DE-SPACED ON THE HOST into
    xH[128 part, 8 chunk, 4 dblk, 7 j, 128 seg] fp16 with d = 128*dblk +
    part and token = 1024*chunk + 8*seg + j: the device never transposes
    anything, the space tokens are never staged (12.5% less HBM traffic),
    and fp16 staging halves it again. Each chunk DMA reads one contiguous
    7 KB descriptor per partition at line rate.
  - Chunk loads ALTERNATE between the two HWDGE rings (SP/ACT) and the
    mid-stream chunks (2,3) and (4,5) are GROUPED into single double-chunk
    DMAs (one contiguous 14 KB descriptor per partition): the tile
    scheduler serializes each ring's per-dma_start fixed cost (~2 us), so
    fewer+bigger mid-stream DMAs cut ring overhead while the fill (0,1)
    and tail (6,7) chunks stay single for minimal latency. Param loads
    lead the HWDGE rings (tiny bias/ones ahead of x on SP, wt ahead of x
    on ACT) - on the SWDGE ring they starve behind the constant HWDGE
    packet traffic and stalled the PE's first matmul by ~17 us.
  - Pooling per chunk is a 4-op fp16 pair-add tree on DVE (contiguous
    128-seg inner runs keep the 16-bit 2-elem/cycle mode) producing
    pooledT[128 dk, 4 blk, 128 seg] directly; the space row is simply
    never added. Scale 1/7 is folded into the staged w.
  - The staged projection weights are CENTERED host-side (wT - rowmean,
    b - mean(b)): the matmul produces proj - mean(proj) exactly (the LN
    mean is linear in the weights), so no mean/bn_stats pass runs on
    device.
  - PE per chunk: K=1 ones x bias_row matmul (bias lives inside the PSUM
    accumulation group - no extra engine hop) + 4 K=128 fp16 matmuls
    pooledT.T @ wTc into PSUM f32.
  - LayerNorm: sum of squares via one ACT Square pass with row-accumulate,
    Sqrt(ss/512+eps) on ACT, reciprocal on DVE (one chunk later, off the
    critical chain), scale-only apply on ACT, fp16 out tile, upcast to f32
    on the host. Out stores are batched per chunk pair on the SP ring
    (after all loads in SP program order - no head-of-line blocking).
  - Emission is software-pipelined in three stages (load+pool / proj+stats
    / apply+store) so late-stage ops never block a later chunk's
    early-stage ops in engine program order.
"""

import numpy as np

B, T, D = 8, 8192, 512
STRIDE = 8
W = T // STRIDE  # 1024
LN_EPS = 1e-5
N_CORES = 8
N_CHUNKS = 8         # 128 segments (= 1024 tokens) per chunk
VALID = STRIDE - 1   # 7 non-space tokens per segment


def _patched_tile_context(tile, mybir, ScopedClock):
    """TileContext whose kernel-tail drain carries no sem waits.

    The walrus build in this container rejects sync-wait commands on Drain
    instructions (setupSyncWait<...NO_STRUCT>: "Too many sync wait commands").
    Stock TileContext parks the global-clock catch-up waits on the SP Drain;
    park them on SP nops (one wait each) instead.
    """

    class PatchedTileContext(tile.TileContext):
        def _drain_and_barrier(self, tick_clock, wait_clock):
            required = ScopedClock({None: tick_clock.global_clock})
            carrier = self.nc.sync.nop(nofuse=True)
            wait_clock.add_sem_waits(carrier.ins, required)
            si = carrier.ins.sync_info
            waits = list(si.on_wait) if si is not None else []
            if len(waits) > 1:
                si.on_wait = waits[:1]
                carrier.ins.sync_info = si
                for w in waits[1:]:
                    extra = self.nc.sync.nop(nofuse=True)
                    extra.ins.sync_info = mybir.SyncInfo(on_wait=[w], on_update=[])
            # The carrier nops run earlier on the same (SP) engine, so the
            # drain transitively waits on everything without carrying waits.
            self.nc.sync.drain()
            self.nc.all_engine_barrier()
            assert self.sems is not None
            popped = self.nc._tile_sem_poison_stack.pop()
            assert popped is self._sem_poison
            self.nc.clear_and_free_semaphores(list(self.sems.allocated().values()))
            self.nc.all_engine_barrier()

    return PatchedTileContext


def _split_multi_waits(nc, mybir):
    """Rewrite the scheduled BIR so no instruction carries more than one sync
    wait (and Drain carries none): the walrus build here rejects them
    (setupSyncWait: "Too many sync wait commands"). Surplus waits move onto
    same-engine InstNoOp carriers placed immediately before the instruction -
    same-engine program order preserves the blocking semantics."""
    n = 0
    for fn in nc.m.functions:
        for bb in fn.blocks:
            changed = False
            new_insts = []
            for inst in bb.instructions:
                si = inst.sync_info
                waits = list(si.on_wait) if si is not None else []
                limit = 0 if inst.opcode == "Drain" else 1
                if len(waits) > limit:
                    changed = True
                    for w in waits[limit:]:
                        n += 1
                        new_insts.append(
                            mybir.InstNoOp(
                                name=f"wsplit_{n}_{inst.name}",
                                engine=inst.engine,
                                sync_info=mybir.SyncInfo(on_wait=[w], on_update=[]),
                                bass_nofuse=True,
                            )
                        )
                    si.on_wait = waits[:limit]
                    inst.sync_info = si
                new_insts.append(inst)
            if changed:
                bb.instructions = new_insts


def _build_bass(apply_gamma_beta: bool, split_waits: bool = True):
    import concourse.bass as bass
    import concourse.mybir as mybir
    import concourse.tile as tile
    from concourse.bass import ts
    from concourse.vector_clock import ScopedClock

    PatchedTileContext = _patched_tile_context(tile, mybir, ScopedClock)
    f32 = mybir.dt.float32
    f16 = mybir.dt.float16

    nc = bass.Bass("TRN2")
    # host-transposed x: [part, chunk, dblk, j, seg] with d = 128*dblk + part,
    # token = 1024*chunk + 8*seg + j (contiguous 128-seg inner runs keep the
    # DVE pooling adds in 16-bit 2x mode)
    x = nc.dram_tensor(
        "x", [128, N_CHUNKS, 4, VALID, 128], f16, kind="ExternalInput"
    )
    # w_proj.T / 7, centered over dout
    wT = nc.dram_tensor("wT", [D, D], f16, kind="ExternalInput")
    bias = nc.dram_tensor("bias", [1, D], f16, kind="ExternalInput")
    ones1 = nc.dram_tensor("ones1", [1, 128], f16, kind="ExternalInput")
    if apply_gamma_beta:
        gammaB = nc.dram_tensor("gammaB", [128, D], f32, kind="ExternalInput")
        betaB = nc.dram_tensor("betaB", [128, D], f32, kind="ExternalInput")
    out = nc.dram_tensor("out", [W, D], f16, kind="ExternalOutput")

    with PatchedTileContext(nc) as tc:
        with (
            tc.tile_pool(name="singles", bufs=1) as singles,
            tc.tile_pool(name="xr_pool", bufs=3) as xr_pool,
            tc.tile_pool(name="t_pool", bufs=2) as t_pool,
            tc.tile_pool(name="uv_pool", bufs=2) as uv_pool,
            tc.tile_pool(name="pmT_pool", bufs=4) as pmT_pool,
            tc.tile_pool(name="sq_pool", bufs=3) as sq_pool,
            tc.tile_pool(name="out_sb", bufs=3) as out_sb,
            tc.tile_pool(name="stat", bufs=6) as stat,
            tc.tile_pool(name="ps_proj", bufs=5, space="PSUM") as ps_proj,
        ):
            # One-time loads ride the HWDGE rings ahead of the x streams:
            # SWDGE singles starve behind constant HWDGE traffic (SDMA
            # round-robins at packet granularity), which stalled the PE's
            # first matmul ~20 us. bias/ones are tiny (SP, ~0.1 us before
            # x chunk 0); wt (512 KB) leads the ACT ring before x chunk 1.
            bias_sb = singles.tile([1, D], f16)
            nc.sync.dma_start(out=bias_sb[:], in_=bias[:, :])
            ones_sb = singles.tile([1, 128], f16)
            nc.sync.dma_start(out=ones_sb[:], in_=ones1[:, :])
            wt_sb = singles.tile([128, 4, D], f16)  # [d_lo, d_hi, dout]
            wt_view = wT[:, :].rearrange("(k p) n -> p k n", p=128)
            nc.scalar.dma_start(out=wt_sb[:, 2:4, :], in_=wt_view[:, 2:4, :])
            eps_sb = singles.tile([128, 1], f32)
            nc.vector.memset(eps_sb[:], LN_EPS)
            # Warm-up: trigger the one-time ACT table load and PE state
            # load early, overlapped with the first x-chunk DMA.
            warm_sb = singles.tile([128, 1], f32)
            nc.scalar.activation(
                out=warm_sb[:],
                in_=eps_sb[:],
                func=mybir.ActivationFunctionType.Identity,
                scale=1.0,
            )
            warm_ps = ps_proj.tile([128, D], f32, name="warm", bufs=1)
            nc.tensor.matmul(
                warm_ps[:], lhsT=ones_sb[:], rhs=bias_sb[:], start=True, stop=True
            )
            if apply_gamma_beta:
                g_sb = singles.tile([128, D], f32)
                nc.scalar.dma_start(out=g_sb[:], in_=gammaB[:, :])
                b_sb = singles.tile([128, D], f32)
                nc.scalar.dma_start(out=b_sb[:], in_=betaB[:, :])

            pmTs = {}
            pps = {}
            rstds = {}

            xrs = {}

            def stage_load(group, eng):
                # Grouped loads: chunks are contiguous in xH, so a 2-chunk
                # group is ONE 14 KB descriptor per partition - half the
                # per-dma_start ring fixed cost. Fill (0,1) and tail (6,7)
                # chunks stay single so their latency is minimal.
                n = len(group)
                xr = xr_pool.tile(
                    [128, n, 4, VALID, 128], f16, name=f"xr{n}"
                )
                c0 = group[0]
                eng.dma_start(out=xr[:], in_=x[:, c0 : c0 + n, :, :, :])
                for i, R in enumerate(group):
                    xrs[R] = (xr, i)

            def stage_pool(R):
                xr, i = xrs.pop(R)
                # Pooling: fp16 pair-add tree on DVE (contiguous 128-seg
                # inner runs -> 2 elem/cycle); the space row is never added.
                with nc.allow_low_precision(reason="fp16 pooling"):
                    t = t_pool.tile([128, 4, 3, 128], f16, name="t")
                    nc.vector.tensor_add(
                        t[:], xr[:, i, :, 0:5:2, :], xr[:, i, :, 1:6:2, :]
                    )
                    uv1 = uv_pool.tile([128, 4, 128], f16, name="uv1")
                    nc.vector.tensor_add(
                        uv1[:], t[:, :, 2, :], xr[:, i, :, 6, :]
                    )
                    uv0 = uv_pool.tile([128, 4, 128], f16, name="uv0")
                    nc.vector.tensor_add(
                        uv0[:], t[:, :, 0, :], t[:, :, 1, :]
                    )
                    pmT = pmT_pool.tile([128, 4, 128], f16, name="pmT")
                    nc.vector.tensor_add(pmT[:], uv0[:], uv1[:])
                pmTs[R] = pmT

            def stage_b1(R):
                pmT = pmTs.pop(R)
                # projection for w-chunk R: psum[seg 128, dout 512]; the
                # bias lives in the accumulation group (K=1 ones x bias).
                pp = ps_proj.tile([128, D], f32, name="pp")
                nc.tensor.matmul(
                    pp[:], lhsT=ones_sb[:], rhs=bias_sb[:], start=True, stop=False
                )
                for k in range(4):
                    nc.tensor.matmul(
                        pp[:],
                        lhsT=pmT[:, k, :],
                        rhs=wt_sb[:, k, :],
                        start=False,
                        stop=(k == 3),
                    )
                # Sum of squares via one ACT Square pass with row-accumulate,
                # then Sqrt; the tiny reciprocal runs on DVE one chunk later
                # (stage_b2) so the ACT->DVE->ACT zig is off the chain.
                sq = sq_pool.tile([128, D], f16, name="sq")
                ss = stat.tile([128, 1], f32, name="ss")
                nc.scalar.activation(
                    out=sq[:],
                    in_=pp[:],
                    func=mybir.ActivationFunctionType.Square,
                    accum_out=ss[:],
                )
                rstd = stat.tile([128, 1], f32, name="rstd")
                nc.scalar.activation(
                    out=rstd[:],
                    in_=ss[:],
                    func=mybir.ActivationFunctionType.Sqrt,
                    bias=eps_sb[:],
                    scale=1.0 / D,
                )
                pps[R] = pp
                rstds[R] = rstd

            ots = {}

            def stage_b2(R):
                pp = pps.pop(R)
                rstd = rstds.pop(R)
                nc.vector.reciprocal(out=rstd[:], in_=rstd[:])
                # Output tiles are batched per chunk pair: one 256-row
                # store per two chunks halves the per-store fixed cost.
                if R % 2 == 0:
                    ot2 = out_sb.tile([128, 2, D], f16, name="ot2")
                    ots[R // 2] = ot2
                else:
                    ot2 = ots[R // 2]
                if apply_gamma_beta:
                    ot32 = out_sb.tile([128, D], f32, name="ot32")
                    nc.scalar.activation(
                        out=ot32[:],
                        in_=pp[:],
                        func=mybir.ActivationFunctionType.Identity,
                        scale=rstd[:],
                    )
                    nc.vector.tensor_mul(out=ot32[:], in0=ot32[:], in1=g_sb[:])
                    nc.vector.tensor_add(
                        out=ot2[:, R % 2, :], in0=ot32[:], in1=b_sb[:]
                    )
                elif R >= 6:
                    nc.vector.tensor_scalar_mul(
                        ot2[:, R % 2, :], pp[:], rstd[:]
                    )
                else:
                    nc.scalar.activation(
                        out=ot2[:, R % 2, :],
                        in_=pp[:],
                        func=mybir.ActivationFunctionType.Identity,
                        scale=rstd[:],
                    )
                if R % 2 == 1:
                    ot2 = ots.pop(R // 2)
                    nc.sync.dma_start(
                        out=out[R // 2 * 256 : R // 2 * 256 + 256, :].rearrange(
                            "(c p) n -> p c n", p=128
                        ),
                        in_=ot2[:],
                    )

            # Software-pipelined emission: grouped loads lead, pooling
            # follows per chunk, proj+stats (b1) runs ~2 chunks behind,
            # apply+store (b2) one more behind.
            S, A = nc.sync, nc.scalar
            schedule = [
                ("l", (0,), S), ("l", (1,), A), ("wt01",), ("l", (2, 3), S),
                ("p", 0), ("l", (4, 5), A), ("p", 1), ("b1", 0),
                ("l", (6,), S), ("p", 2), ("b1", 1), ("l", (7,), A),
                ("b2", 0), ("p", 3), ("b1", 2), ("b2", 1),
                ("p", 4), ("b1", 3), ("b2", 2),
                ("p", 5), ("b1", 4), ("b2", 3),
                ("p", 6), ("b1", 5), ("b2", 4),
                ("p", 7), ("b1", 6), ("b2", 5),
                ("b1", 7), ("b2", 6), ("b2", 7),
            ]
            for item in schedule:
                if item[0] == "l":
                    stage_load(item[1], item[2])
                elif item[0] == "wt01":
                    nc.sync.dma_start(
                        out=wt_sb[:, 0:2, :], in_=wt_view[:, 0:2, :]
                    )
                elif item[0] == "p":
                    stage_pool(item[1])
                elif item[0] == "b1":
                    stage_b1(item[1])
                else:
                    stage_b2(item[1])

    if split_waits:
        _split_multi_waits(nc, mybir)
    return nc


def _stage_inputs(inputs) -> tuple[bool, list[dict]]:
    """Host-side staging: transposed fp16 x per core + replicated params."""
    x = np.asarray(inputs["x"], dtype=np.float32)
    w = np.asarray(inputs["w_proj"], dtype=np.float32)
    b = np.asarray(inputs["b_proj"], dtype=np.float32)
    gamma = np.asarray(inputs["gamma"], dtype=np.float32)
    beta = np.asarray(inputs["beta"], dtype=np.float32)
    assert x.shape == (B, T, D), x.shape

    apply_gb = not (np.all(gamma == 1.0) and np.all(beta == 0.0))
    # Center the projection over dout so the device matmul yields
    # proj - mean(proj) directly (mean is linear in the weights).
    wt = (w.T / VALID).astype(np.float64)
    wtc = wt - wt.mean(axis=1, keepdims=True)
    bc = b.astype(np.float64) - b.astype(np.float64).mean()
    common = {
        "wT": np.ascontiguousarray(wtc).astype(np.float16),
        "bias": np.ascontiguousarray(bc.reshape(1, D)).astype(np.float16),
        "ones1": np.ones((1, 128), dtype=np.float16),
    }
    if apply_gb:
        common["gammaB"] = np.ascontiguousarray(
            np.broadcast_to(gamma.reshape(1, D), (128, D))
        )
        common["betaB"] = np.ascontiguousarray(
            np.broadcast_to(beta.reshape(1, D), (128, D))
        )
    x16 = x.astype(np.float16)
    # xH[p, c, k, j, s] = x[c*1024 + 8*s + j, 128*k + p], j < 7 only
    # (space tokens are never staged).
    in_maps = []
    for i in range(N_CORES):
        xh = np.ascontiguousarray(
            x16[i]
            .reshape(N_CHUNKS, 128, STRIDE, 4, 128)[:, :, :VALID, :, :]
            .transpose(4, 0, 3, 2, 1)
        )
        in_maps.append({"x": xh, **common})
    return apply_gb, in_maps


def kernel(**inputs) -> np.ndarray:
    from concourse.bass_utils import run_bass_kernel_spmd

    apply_gb, in_maps = _stage_inputs(inputs)
    nc = _build_bass(apply_gb)
    res = run_bass_kernel_spmd(nc, in_maps, core_ids=list(range(N_CORES)))
    return np.stack(
        [res.results[i]["out"].astype(np.float32) for i in range(N_CORES)], axis=0
    )


if __name__ == "__main__":
    rng = np.random.default_rng(0)
    demo = {
        "x": rng.standard_normal((B, T, D), dtype=np.float32),
        "input_ids": np.zeros((B, T), dtype=np.int64),
        "w_proj": rng.standard_normal((D, D), dtype=np.float32) / np.sqrt(D),
        "b_proj": (rng.standard_normal(D) * 0.01).astype(np.float32),
        "gamma": np.ones(D, dtype=np.float32),
        "beta": np.zeros(D, dtype=np.float32),
    }
    out = kernel(**demo)
    print(out.shape, out.dtype, float(np.abs(out).mean()))
